# revision 48
# baseline (speedup 1.0000x reference)
"""Trainium2 Bass kernel for nn_DistanceTokenEncoder.

Strategy (8-core SPMD, row-sharded):
  - Each core owns NI=48 token rows i. Pairs per core: 4 channels x 48 x 384.
  - Feature-major layout: activations live as x^T [feature, pair] so the
    Transition matmuls need no transposes.
  - LayerNorm is folded into the weights on the host: ln_w merges into w1/w2,
    the mean subtraction becomes column-centered weights (w - colsum(w)/257),
    and the rstd scaling is applied post-matmul on device. Sum/sumsq per pair
    come from ones-matmuls that broadcast the stats across all 128 partitions.
  - Gaussian smearing (d - offset_g)^2 is produced directly by a K=3 matmul
    with rhs rows [d; d^2; 1], then a single ACT Exp.
  - sqrt / rsqrt / sigmoid are synthesized from Ln+Exp so the whole kernel
    uses one activation table set (natural_log_exp_and_others).
  - Main-loop matmul operands are bf16 (full-rate PE); accumulation and the
    stats chain stay fp32 in PSUM/SBUF.
  - Output is written channel-blocked [t, o, 4, F]; the host interleaves to
    the final [i, j, o*4+c] layout while unsharding.
"""

import numpy as np
import ml_dtypes
from contextlib import ExitStack

import concourse.bacc as bacc
import concourse.tile as tile
from concourse import mybir
from concourse.bass_utils import run_bass_kernel_spmd

# The activation-table-load pass picks the first set containing each function,
# which thrashes between exp_and_others and natural_log (~2.7us per switch,
# hundreds of switches). Every function this kernel uses lives in
# natural_log_exp_and_others, so restrict the selectable sets to that one
# (other entries stay in place so act_func_set_id indices remain valid).
_orig_get_tables = bacc.get_activation_tables


def _patched_get_tables(module_arch):
    tabs = _orig_get_tables(module_arch)
    keep = "natural_log_exp_and_others"
    return {nm: (fns if nm == keep else set()) for nm, fns in tabs.items()}


bacc.get_activation_tables = _patched_get_tables

AFT = mybir.ActivationFunctionType
FP = mybir.dt.float32
HF = mybir.dt.float16
NPHF = np.float16

# problem constants (hardcoded per harness contract)
N, Z, G, A4 = 384, 128, 128, 1536
M_CORES = 8
NI = N // M_CORES            # 48 token rows per core
NP = NI * N                  # 18432 pairs per (core, channel)
F = 512                      # pairs per inner tile
NT = NP // F                 # 36 tiles
NF = G + 1 + Z               # 257 features
START, STOP = 0.0, 2.0
COEFF = -0.5 / ((STOP - START) / (G - 1)) ** 2
LN_EPS = 1e-5
RNF = 1.0 / np.sqrt(NF)      # 1/sqrt(257)


def build_nc(use_bias: bool):
    nc = bacc.Bacc()

    rpeT = nc.declare_dram_parameter("rpeT", [Z, NP], HF, False)
    R_all_d = nc.declare_dram_parameter("R_all", [5, 4 * N], FP, False)
    Q_co_d = nc.declare_dram_parameter("Q_co", [5, 4 * NI], FP, False)
    w1_d = nc.declare_dram_parameter("w1h", [NF, Z], HF, False)
    w2_d = nc.declare_dram_parameter("w2h", [NF, Z], HF, False)
    w3_d = nc.declare_dram_parameter("w3b", [Z, 32], HF, False)
    glt_d = nc.declare_dram_parameter("glt", [3, G], FP, False)
    dmask_d = nc.declare_dram_parameter("dmask", [NI, N], FP, False)
    if use_bias:
        bb1_d = nc.declare_dram_parameter("bb1", [Z, 1], FP, False)
        bb2_d = nc.declare_dram_parameter("bb2", [Z, 1], FP, False)
    out_d = nc.declare_dram_parameter("out", [NT, 32, 4 * F], FP, True)
    # DRAM scratch for per-channel rows used by the main loop:
    # fp32 [d; d^2; ones] feeds the K=3 squared-distance matmul (the
    # (d-o)^2 cancellation needs fp32), bf16 [d; d^2] feeds the K=1
    # feature/stat matmuls.
    dd_scr = nc.dram_tensor("dd_scr", [4, 3, NP], FP)
    dd_hfs = nc.dram_tensor("dd_hfs", [4, 2, NP], HF)

    with tile.TileContext(nc) as tc, ExitStack() as ctx:
        const = ctx.enter_context(tc.tile_pool(name="const", bufs=1))
        wk = ctx.enter_context(tc.tile_pool(name="wk", bufs=1))
        mt = ctx.enter_context(tc.tile_pool(name="mt", bufs=2))
        stg = ctx.enter_context(tc.tile_pool(name="stg", bufs=2))
        ph_ctx = ExitStack()
        ph = ph_ctx.enter_context(tc.tile_pool(name="ph", bufs=1, space="PSUM"))

        # ---------------- phase 0: constants + weights ----------------
        rpeT_sb = const.tile([Z, NP], HF, tag="rpeT")
        CH = NP // 6
        for k in range(6):
            nc.sync.dma_start(
                out=rpeT_sb[:, k * CH:(k + 1) * CH],
                in_=rpeT[:, k * CH:(k + 1) * CH],
            )

        glt_sb = const.tile([3, G], FP, tag="glt")
        nc.sync.dma_start(out=glt_sb[:], in_=glt_d[:])
        dmask_sb = const.tile([NI, N], FP, tag="dmask")
        nc.sync.dma_start(out=dmask_sb[:], in_=dmask_d[:])

        # pre-folded, column-centered weights (bf16), split by K chunk.
        # Feature order is [dg 0:128, d 128, rpe 129:257]: chunk a = gaussian
        # rows, chunk b = rpe rows, chunk c = the single raw-distance row.
        wbf = {}
        for nm, wd in (("w1", w1_d), ("w2", w2_d)):
            a = const.tile([128, Z], HF, tag=f"{nm}a")
            b = const.tile([128, Z], HF, tag=f"{nm}b")
            c_ = const.tile([1, Z], HF, tag=f"{nm}c")
            nc.sync.dma_start(out=a[:], in_=wd[0:G, :])
            nc.sync.dma_start(out=b[:], in_=wd[G + 1:NF, :])
            nc.sync.dma_start(out=c_[:], in_=wd[G:G + 1, :])
            wbf[nm] = (a, b, c_)
        w3_sb = const.tile([Z, 32], HF, tag="w3")
        nc.sync.dma_start(out=w3_sb[:], in_=w3_d[:])

        bcols = {}
        if use_bias:
            for nm, bd in (("w1", bb1_d), ("w2", bb2_d)):
                bb = const.tile([Z, 1], FP, tag=f"bb{nm}")
                nc.sync.dma_start(out=bb[:], in_=bd[:])
                bcols[nm] = bb

        qones = const.tile([128, 128], HF, tag="qones")
        nc.vector.memset(qones[:], 1.0)
        lneps_col = const.tile([128, 1], FP, tag="lneps")
        nc.vector.memset(lneps_col[:], LN_EPS)
        eps20_col = const.tile([128, 1], FP, tag="eps20")
        nc.vector.memset(eps20_col[:], 1e-20)
        ones48 = const.tile([NI, N], FP, tag="ones48")
        nc.vector.memset(ones48[:], 1.0)

        # ---------------- phase 0b: pair-matmul operands ----------------
        # R_all [5, 4N] rows [-2x, -2y, -2z, 1, |p|^2] and Q_co
        # [5, 4*NI] rows [x, y, z, |p|^2, 1] are host-computed in float64
        # (the d^2 gram cancellation needs better-than-fp32 inputs).
        R_all = const.tile([5, 4 * N], FP, tag="R_all")
        nc.sync.dma_start(out=R_all[:], in_=R_all_d[:])
        Q_co = const.tile([5, 4 * NI], FP, tag="Q_co")
        nc.sync.dma_start(out=Q_co[:], in_=Q_co_d[:])

        # ---------------- phase 1: distances per channel ----------------
        for c in range(4):
            pd2 = ph.tile([NI, N], FP, tag="pd2")
            nc.tensor.matmul(
                out=pd2[:],
                lhsT=Q_co[:, c * NI:(c + 1) * NI],
                rhs=R_all[:, c * N:(c + 1) * N],
                start=True, stop=True,
            )
            d2a = wk.tile([NI, N], FP, tag="d2a")
            nc.vector.tensor_scalar_max(out=d2a[:], in0=pd2[:], scalar1=0.0)
            d2m = wk.tile([NI, N], FP, tag="d2m")
            nc.vector.tensor_mul(out=d2m[:], in0=d2a[:], in1=dmask_sb[:])
            l2 = wk.tile([NI, N], FP, tag="l2")
            nc.scalar.activation(out=l2[:], in_=d2m[:], func=AFT.Ln,
                                 bias=eps20_col[0:NI, :])
            d0 = wk.tile([NI, N], FP, tag="d0")
            nc.scalar.activation(out=d0[:], in_=l2[:], func=AFT.Exp, scale=0.5)
            # one Newton step d = (d0 + d2/d0)/2 — the ACT Ln table is only
            # ~400 ULP and the gaussian needs d to ~1e-6 relative
            rcp = wk.tile([NI, N], FP, tag="rcp")
            nc.vector.reciprocal(out=rcp[:], in_=d0[:])
            tq = wk.tile([NI, N], FP, tag="tq")
            nc.vector.tensor_mul(out=tq[:], in0=d2m[:], in1=rcp[:])
            dsb = wk.tile([NI, N], FP, tag="dsb")
            nc.vector.tensor_add(out=dsb[:], in0=d0[:], in1=tq[:])
            nc.vector.tensor_scalar_mul(out=dsb[:], in0=dsb[:], scalar1=0.5)
            d_bfc = wk.tile([NI, N], HF, tag="d_bfc")
            nc.vector.tensor_copy(out=d_bfc[:], in_=dsb[:])
            d2_bfc = wk.tile([NI, N], HF, tag="d2_bfc")
            nc.vector.tensor_copy(out=d2_bfc[:], in_=d2m[:])

            nc.sync.dma_start(
                out=dd_scr[c, 0, :].rearrange("(i j) -> i j", j=N), in_=dsb[:]
            )
            nc.sync.dma_start(
                out=dd_scr[c, 1, :].rearrange("(i j) -> i j", j=N), in_=d2m[:]
            )
            nc.sync.dma_start(
                out=dd_scr[c, 2, :].rearrange("(i j) -> i j", j=N), in_=ones48[:]
            )
            nc.sync.dma_start(
                out=dd_hfs[c, 0, :].rearrange("(i j) -> i j", j=N), in_=d_bfc[:]
            )
            nc.sync.dma_start(
                out=dd_hfs[c, 1, :].rearrange("(i j) -> i j", j=N), in_=d2_bfc[:]
            )

        # ---------------- phase 2: main loop ----------------
        ph_ctx.close()  # release phase-0/1 PSUM banks
        pm_sq = ctx.enter_context(tc.tile_pool(name="pm_sq", bufs=2, space="PSUM"))
        pm_u = ctx.enter_context(tc.tile_pool(name="pm_u", bufs=1, space="PSUM"))
        pm_s = ctx.enter_context(tc.tile_pool(name="pm_s", bufs=1, space="PSUM"))
        pm_o = ctx.enter_context(tc.tile_pool(name="pm_o", bufs=2, space="PSUM"))
        w1a, w1b, w1c = wbf["w1"]
        w2a, w2b, w2c = wbf["w2"]
        for t in range(NT):
            sl = slice(t * F, (t + 1) * F)
            rpe_sl = rpeT_sb[:, sl]
            rpe2 = mt.tile([Z, F], HF, tag="rpe2")
            nc.vector.tensor_mul(out=rpe2[:], in0=rpe_sl, in1=rpe_sl)
            stage = stg.tile([32, 4 * F], FP, tag="stage")
            for c in range(4):
                dd = mt.tile([3, F], FP, tag="dd")
                nc.sync.dma_start(out=dd[:], in_=dd_scr[c, :, sl])
                dr = mt.tile([1, F], HF, tag="dr")
                nc.sync.dma_start(out=dr[:], in_=dd_hfs[c, 0, sl])
                d2r = mt.tile([1, F], HF, tag="d2r")
                nc.sync.dma_start(out=d2r[:], in_=dd_hfs[c, 1, sl])
                ddd = dd[0:3, :]
                d_row = dr[0:1, :]
                d2_row = d2r[0:1, :]

                psq = pm_sq.tile([G, F], FP, tag="sq")
                nc.tensor.matmul(out=psq[:], lhsT=glt_sb[:], rhs=ddd,
                                 start=True, stop=True)
                dg = mt.tile([G, F], HF, tag="dg")
                nc.scalar.activation(out=dg[:], in_=psq[:], func=AFT.Exp,
                                     scale=float(COEFF))
                dg2 = mt.tile([G, F], HF, tag="dg2")
                nc.gpsimd.tensor_mul(out=dg2[:], in0=dg[:], in1=dg[:])

                pU1 = pm_u.tile([Z, F], FP, tag="U1")
                nc.tensor.matmul(out=pU1[:], lhsT=w1a[:], rhs=dg[:],
                                 start=True, stop=False)
                nc.tensor.matmul(out=pU1[:], lhsT=w1b[:], rhs=rpe_sl,
                                 start=False, stop=False)
                nc.tensor.matmul(out=pU1[:], lhsT=w1c[:], rhs=d_row,
                                 start=False, stop=True)
                pU2 = pm_u.tile([Z, F], FP, tag="U2")
                nc.tensor.matmul(out=pU2[:], lhsT=w2a[:], rhs=dg[:],
                                 start=True, stop=False)
                nc.tensor.matmul(out=pU2[:], lhsT=w2b[:], rhs=rpe_sl,
                                 start=False, stop=False)
                nc.tensor.matmul(out=pU2[:], lhsT=w2c[:], rhs=d_row,
                                 start=False, stop=True)

                ps = pm_s.tile([128, F], FP, tag="s")
                nc.tensor.matmul(out=ps[:], lhsT=qones[:], rhs=dg[:],
                                 start=True, stop=False)
                nc.tensor.matmul(out=ps[:], lhsT=qones[:], rhs=rpe_sl,
                                 start=False, stop=False)
                nc.tensor.matmul(out=ps[:], lhsT=qones[0:1, :], rhs=d_row,
                                 start=False, stop=True)
                pq = pm_s.tile([128, F], FP, tag="q")
                nc.tensor.matmul(out=pq[:], lhsT=qones[:], rhs=dg2[:],
                                 start=True, stop=False)
                nc.tensor.matmul(out=pq[:], lhsT=qones[:], rhs=rpe2[:],
                                 start=False, stop=False)
                nc.tensor.matmul(out=pq[:], lhsT=qones[0:1, :], rhs=d2_row,
                                 start=False, stop=True)

                # rstd = exp(-0.5*ln((q - (s/sqrt(NF))^2)/NF + eps))
                wsq = mt.tile([128, F], FP, tag="wsq")
                nc.scalar.activation(out=wsq[:], in_=ps[:], func=AFT.Square,
                                     scale=float(RNF))
                u = mt.tile([128, F], FP, tag="u")
                nc.vector.tensor_sub(out=u[:], in0=pq[:], in1=wsq[:])
                lu = mt.tile([128, F], FP, tag="lu")
                nc.scalar.activation(out=lu[:], in_=u[:], func=AFT.Ln,
                                     bias=lneps_col[:], scale=1.0 / NF)
                rstd = mt.tile([128, F], FP, tag="rstd")
                nc.scalar.activation(out=rstd[:], in_=lu[:], func=AFT.Exp,
                                     scale=-0.5)

                A1 = mt.tile([Z, F], HF, tag="A1")
                nc.vector.tensor_mul(out=A1[:], in0=pU1[:], in1=rstd[:])
                A2 = mt.tile([Z, F], HF, tag="A2")
                nc.vector.tensor_mul(out=A2[:], in0=pU2[:], in1=rstd[:])
                if use_bias:
                    y1 = mt.tile([Z, F], HF, tag="y1")
                    nc.vector.tensor_scalar_add(out=y1[:], in0=A1[:],
                                                scalar1=bcols["w1"][:])
                    y2 = mt.tile([Z, F], HF, tag="y2")
                    nc.vector.tensor_scalar_add(out=y2[:], in0=A2[:],
                                                scalar1=bcols["w2"][:])
                else:
                    y1, y2 = A1, A2

                # silu(y1)*y2 = y1*y2*exp(-ln(1+exp(-y1)))
                e = mt.tile([Z, F], HF, tag="e")
                nc.scalar.activation(out=e[:], in_=y1[:], func=AFT.Exp,
                                     scale=-1.0)
                spl = mt.tile([Z, F], HF, tag="spl")
                nc.scalar.activation(out=spl[:], in_=e[:], func=AFT.Ln, bias=1.0)
                sg = mt.tile([Z, F], HF, tag="sg")
                nc.scalar.activation(out=sg[:], in_=spl[:], func=AFT.Exp,
                                     scale=-1.0)

                m = mt.tile([Z, F], HF, tag="m")
                nc.vector.tensor_mul(out=m[:], in0=y1[:], in1=y2[:])
                h = mt.tile([Z, F], HF, tag="h")
                nc.vector.tensor_mul(out=h[:], in0=m[:], in1=sg[:])

                po = pm_o.tile([32, F], FP, tag="o")
                nc.tensor.matmul(out=po[:], lhsT=w3_sb[:], rhs=h[:],
                                 start=True, stop=True)
                nc.vector.tensor_copy(out=stage[:, c * F:(c + 1) * F], in_=po[:])
            nc.sync.dma_start(out=out_d[t], in_=stage[:])

    nc.compile()
    return nc


_CACHE = {}


def _get_nc(use_bias: bool):
    if use_bias not in _CACHE:
        _CACHE[use_bias] = build_nc(use_bias)
    return _CACHE[use_bias]


def prepare_in_maps(inputs):
    rpe = np.ascontiguousarray(
        np.asarray(inputs["relative_position_encoding"], np.float32)[0]
    )
    t2b = np.asarray(inputs["token_to_bb4_atoms"], np.float32)[0]
    coords = np.ascontiguousarray(np.asarray(inputs["coords"], np.float32))[0]
    lnw = np.asarray(inputs["ln_w"], np.float32).reshape(NF)
    lnb = np.asarray(inputs["ln_b"], np.float32).reshape(NF)
    w1 = np.asarray(inputs["w1"], np.float32)
    w2 = np.asarray(inputs["w2"], np.float32)
    w3 = np.asarray(inputs["w3"], np.float32)

    # fold LayerNorm affine into the weights; center columns for the
    # mean subtraction (x - mu) @ w' == x @ (w' - colsum(w')/NF)
    w1p = lnw[:, None] * w1
    w2p = lnw[:, None] * w2
    w1h = (w1p - w1p.sum(0)[None, :] / NF).astype(NPHF)
    w2h = (w2p - w2p.sum(0)[None, :] / NF).astype(NPHF)
    bb1 = (lnb @ w1).astype(np.float32).reshape(Z, 1)
    bb2 = (lnb @ w2).astype(np.float32).reshape(Z, 1)
    use_bias = bool(np.any(lnb != 0))

    # backbone-atom coordinates in float64 — the d^2 gram-matrix trick
    # (|pi|^2 + |pj|^2 - 2 pi.pj) cancels catastrophically otherwise
    r64 = t2b.astype(np.float64) @ coords.astype(np.float64)  # [m, 3]
    n2_64 = (r64 * r64).sum(1)                                # [m]
    m_order_full = np.array([j * 4 + c for c in range(4) for j in range(N)])
    R_all = np.concatenate([
        -2.0 * r64[m_order_full].T,
        np.ones((1, 4 * N)),
        n2_64[None, m_order_full],
    ]).astype(np.float32)

    off = np.linspace(START, STOP, G).astype(np.float32)
    glt = np.ascontiguousarray(
        np.stack([-2.0 * off, np.ones(G, np.float32), off * off])
    ).astype(np.float32)

    in_maps = []
    for core in range(M_CORES):
        i0 = core * NI
        m_order_core = np.array(
            [(i0 + il) * 4 + c for c in range(4) for il in range(NI)]
        )
        mask = np.ones((NI, N), np.float32)
        mask[np.arange(NI), i0 + np.arange(NI)] = 0.0
        Q_co = np.concatenate([
            r64[m_order_core].T,
            n2_64[None, m_order_core],
            np.ones((1, 4 * NI)),
        ]).astype(np.float32)
        im = {
            "rpeT": np.ascontiguousarray(
                rpe[i0:i0 + NI].reshape(NP, Z).T.astype(NPHF)
            ),
            "R_all": R_all,
            "Q_co": Q_co,
            "w1h": w1h,
            "w2h": w2h,
            "w3b": np.ascontiguousarray(w3.astype(NPHF)),
            "glt": glt,
            "dmask": mask,
        }
        if use_bias:
            im["bb1"] = bb1
            im["bb2"] = bb2
        in_maps.append(im)
    return in_maps, use_bias


def unshard(results):
    full = np.zeros((N, N, 128), np.float32)
    for core in range(M_CORES):
        i0 = core * NI
        a = results[core]["out"].reshape(NT, 32, 4, F)
        full[i0:i0 + NI] = (
            a.transpose(0, 3, 1, 2).reshape(NP, 128).reshape(NI, N, 128)
        )
    return full[None]


def kernel(**inputs):
    in_maps, use_bias = prepare_in_maps(inputs)
    nc = _get_nc(use_bias)
    res = run_bass_kernel_spmd(nc, in_maps, list(range(M_CORES)))
    return unshard(res.results)
